# revision 1
# baseline (speedup 1.0000x reference)
"""CARAFE kernel for Trainium2 (8 NeuronCores, batch-parallel).

Reference computation per image:
  R = relu(conv1x1(x, w_compress, b_compress))          [48, 128, 128]
  E = conv3x3(R, w_encoder, b_encoder, pad=1)           [100, 128, 128]
  Y = softmax over k of E.reshape(4, 25, H, W)          (s, k, h, w)
  out[s,c,h,w] = sum_k Y[s,k,h,w] * xpad[c, h+dy, w+dx] (k=(dy,dx), 5x5, pad 2)
  pixel-shuffle: out_ref[s*16 + c//4, 2h + (c//2)%2, 2w + c%2] = out[s,c,h,w]

Mapping:
  - conv1x1 / conv3x3 / softmax-denominator: TensorE matmuls (channel-major),
    exp on ScalarE.  Biases folded in via a constant ones row (K=65 / K=49).
  - softmax normalization folded into the F-transpose epilogue on ScalarE.
  - The per-pixel weighted patch sum runs on VectorE in pixel-major layout
    [128 h-partitions, (c, w) free]: one mult + one add per (s, k) tap with
    the per-pixel weight broadcast along c via a free-dim step-0 AP.
    dy taps select one of five partition-shifted copies of X^T (built by
    DMA; compute engines cannot shift partitions), dx taps are free-dim
    offsets into a w-padded buffer (zero padding gives the conv edge
    semantics for free).
"""

import sys

import numpy as np

sys.path.insert(0, "/opt/trn_rl_repo")

import concourse.bass as bass
import concourse.mybir as mybir
import concourse.tile as tile
from concourse import bacc
from concourse.masks import make_identity

F32 = mybir.dt.float32

H = 128
W = 128
C = 64
M = 48  # compressed channels
S2 = 4  # scale_factor**2
K2 = 25  # k_up**2
SK = 100
HW = H * W
WPAD = W + 4  # w-padded pixel-major buffers
N_CORES = 8


def _ap(t, extra_off, dims):
    """Raw AP on a tile handle `t` with free-offset `extra_off` (elements)
    and explicit [step, count] dims (dims[0] is the partition dim)."""
    base = t[:]
    return bass.AP(tensor=base.tensor, offset=base.offset + extra_off, ap=dims)


class _Pool:
    """Manually scoped tile pool."""

    def __init__(self, tc, **kw):
        self._cm = tc.tile_pool(**kw)
        self.pool = self._cm.__enter__()
        self._n = 0

    def tile(self, *a, tag=None, **kw):
        self._n += 1
        t = tag or f"t{self._n}"
        return self.pool.tile(*a, tag=t, name=t, **kw)

    def close(self):
        self._cm.__exit__(None, None, None)


def build_program(debug=False, reps=1):
    nc = bacc.Bacc("TRN2", target_bir_lowering=False, debug=False)

    xin = nc.dram_tensor("xin", [C, HW], F32, kind="ExternalInput")
    w1t = nc.dram_tensor("w1t", [C + 1, M], F32, kind="ExternalInput")
    wet = nc.dram_tensor("wet", [M + 1, 9 * SK], F32, kind="ExternalInput")
    sones = nc.dram_tensor("sones", [SK, S2], F32, kind="ExternalInput")
    onesr = nc.dram_tensor("onesr", [1, 130 * 130], F32, kind="ExternalInput")
    zer = nc.dram_tensor("zer", [2, C * WPAD], F32, kind="ExternalInput")
    out = nc.dram_tensor("out", [C, 2 * H, 2 * W], F32, kind="ExternalOutput")
    dbg = {}
    if debug:
        dbg["R"] = nc.dram_tensor("dbgR", [M, HW], F32, kind="ExternalOutput")
        dbg["F"] = nc.dram_tensor("dbgF", [SK, HW], F32, kind="ExternalOutput")
        dbg["FR"] = nc.dram_tensor("dbgFR", [128, SK * W], F32, kind="ExternalOutput")
        dbg["XT"] = nc.dram_tensor("dbgXT", [128, C * WPAD], F32, kind="ExternalOutput")

    with tile.TileContext(nc) as tc:
        cp = _Pool(tc, name="consts", bufs=1)
        pp = _Pool(tc, name="persist", bufs=1)

        w1t_sb = cp.tile([C + 1, M], F32)
        nc.sync.dma_start(w1t_sb[:], w1t.ap())
        wet_sb = cp.tile([M + 1, 9 * SK], F32)
        nc.sync.dma_start(wet_sb[:], wet.ap())
        sones_sb = cp.tile([SK, S2], F32)
        nc.sync.dma_start(sones_sb[:], sones.ap())
        ident = cp.tile([128, 128], F32)
        make_identity(nc, ident[:])
        rzbuf = pp.tile([128, S2 * W], F32)
        xt_base = pp.tile([128, C * WPAD], F32)

        for _rep in range(reps):
            # ---- load x (+ ones row) ----
            px = _Pool(tc, name="px", bufs=1)
            x_aug = px.tile([C + 1, HW], F32)
            nc.sync.dma_start(x_aug[0:C, :], xin.ap())
            nc.sync.dma_start(
                _ap(x_aug, C * HW, [[HW, 1], [1, HW]]), onesr.ap()[:, 0:HW]
            )

            # ---- X^T via PE transpose -> XT_base [128(h), (c, WPAD)] ----
            nc.vector.memset(xt_base[:], 0.0)
            psX = _Pool(tc, name="psX", bufs=2, space="PSUM")
            for w in range(W):
                psx = psX.tile([128, C], F32, tag="psx")
                nc.tensor.transpose(
                    psx[:], _ap(x_aug, w, [[HW, C], [W, H]]), ident[0:C, 0:C]
                )
                nc.scalar.copy(
                    _ap(xt_base, 2 + w, [[C * WPAD, 128], [WPAD, C]]), psx[:]
                )
            psX.close()
            if debug:
                nc.sync.dma_start(dbg["XT"].ap(), xt_base[:])

            # ---- pass 1 (banded): conv1x1 -> relu -> r_band; conv3x3 -> exp -> f_dram; Z ----
            f_dram = nc.dram_tensor(f"fstage{_rep}", [SK, HW], F32, kind="Internal")
            BH = 32  # band height
            RB = BH + 2  # rows held per band (1-halo each side)
            RBF = RB * 130
            pband = _Pool(tc, name="pband", bufs=2)
            psA = _Pool(tc, name="psA", bufs=2, space="PSUM")
            psB = _Pool(tc, name="psB", bufs=2, space="PSUM")
            psBsb = _Pool(tc, name="psBsb", bufs=2)

            def conv1x1_rows(r_band, h0, nrows, loc0):
                """conv1x1+relu for image rows [h0, h0+nrows) into band-local row loc0."""
                ps1 = psA.tile([M, 512], F32, tag="ps1")
                nc.tensor.matmul(
                    ps1[:, 0 : nrows * W],
                    w1t_sb[:],
                    x_aug[:, h0 * W : (h0 + nrows) * W],
                    start=True,
                    stop=True,
                )
                nc.scalar.activation(
                    _ap(r_band, loc0 * 130 + 1, [[RBF, M], [130, nrows], [1, W]]),
                    ps1[:, 0 : nrows * W],
                    mybir.ActivationFunctionType.Relu,
                )

            for b in range(4):
                r_band = pband.tile([M + 1, RBF], F32, tag="rband")
                nc.gpsimd.memset(r_band[:], 0.0)
                nc.sync.dma_start(
                    _ap(r_band, M * RBF, [[RBF, 1], [1, RBF]]), onesr.ap()[:, 0:RBF]
                )
                # band covers image rows 32b-1 .. 32b+32 at band-local rows 0..33
                if b > 0:
                    conv1x1_rows(r_band, 32 * b - 1, 1, 0)
                for j in range(8):
                    conv1x1_rows(r_band, 32 * b + 4 * j, 4, 1 + 4 * j)
                if b < 3:
                    conv1x1_rows(r_band, 32 * b + 32, 1, 33)
                for j in range(8):
                    ps2 = psB.tile([SK, 512], F32, tag="ps2")
                    for t in range(9):
                        ty, tx = divmod(t, 3)
                        nc.tensor.matmul(
                            ps2[:],
                            wet_sb[:, t * SK : (t + 1) * SK],
                            _ap(
                                r_band,
                                (4 * j + ty) * 130 + tx,
                                [[RBF, M + 1], [130, 4], [1, W]],
                            ),
                            start=(t == 0),
                            stop=(t == 8),
                        )
                    fc = psBsb.tile([SK, 512], F32, tag="fc")
                    nc.scalar.activation(
                        fc[:], ps2[:], mybir.ActivationFunctionType.Exp
                    )
                    n = 8 * b + j
                    nc.sync.dma_start(
                        f_dram.ap()[:, n * 512 : (n + 1) * 512], fc[:]
                    )
                    psz = psB.tile([S2, 512], F32, tag="psz")
                    nc.tensor.matmul(
                        psz[:], sones_sb[:], fc[:], start=True, stop=True
                    )
                    zc = psBsb.tile([S2, 512], F32, tag="zc")
                    nc.scalar.copy(zc[:], psz[:])
                    # scatter Z into rzbuf [128(h), (s, w)]: rows 4n..4n+3
                    for s in range(S2):
                        nc.sync.dma_start(
                            _ap(
                                rzbuf,
                                4 * n * (S2 * W) + s * W,
                                [[S2 * W, 4], [1, W]],
                            ),
                            _ap(zc, s * 512, [[512, 1], [W, 4], [1, W]]),
                        )
            psBsb.close()
            psB.close()
            psA.close()
            pband.close()
            px.close()

            nc.vector.reciprocal(rzbuf[:], rzbuf[:])

            # ---- pass 2: reload F, transposes ----
            pfr = _Pool(tc, name="pfr", bufs=1)
            fr = pfr.tile([128, SK * W], F32)
            pf = _Pool(tc, name="pf", bufs=1)
            f_sb = pf.tile([SK, HW], F32)
            nc.sync.dma_start(f_sb[:], f_dram.ap())
            if debug:
                nc.sync.dma_start(dbg["F"].ap(), f_sb[:])


            # ---- F^T transposes + softmax-normalize -> FR [128(h), (sk, w)] ----
            psF = _Pool(tc, name="psF", bufs=2, space="PSUM")
            for w in range(W):
                pst = psF.tile([128, SK], F32, tag="pst")
                nc.tensor.transpose(
                    pst[:], _ap(f_sb, w, [[HW, SK], [W, H]]), ident[0:SK, 0:SK]
                )
                for s in range(S2):
                    nc.scalar.activation(
                        _ap(fr, (s * K2) * W + w, [[SK * W, 128], [W, K2]]),
                        pst[:, s * K2 : (s + 1) * K2],
                        mybir.ActivationFunctionType.Copy,
                        scale=rzbuf[:, s * W + w : s * W + w + 1],
                    )
            psF.close()
            pf.close()
            if debug:
                nc.sync.dma_start(dbg["FR"].ap(), fr[:])

            # ---- per-pixel patch sum on VectorE ----
            WHF = W // 2  # 64 output w per half
            XF = C * (WHF + 4)
            xtp = _Pool(tc, name="xtd", bufs=2)
            accp = _Pool(tc, name="acc", bufs=1)
            tmpp = _Pool(tc, name="tmp", bufs=1)
            acc2p = _Pool(tc, name="acc2", bufs=1)
            for half in range(2):
                for s in range(S2):
                    acc = accp.tile([128, C * WHF], F32, tag="acc")
                    for dy in range(-2, 3):
                        xtd = xtp.tile([128, XF], F32, tag="xtd")
                        p0, p1 = max(0, -dy), 128 - max(0, dy)
                        # body: partition-shifted, w-windowed copy of XT_base
                        nc.sync.dma_start(
                            _ap(xtd, p0 * XF, [[XF, p1 - p0], [1, XF]]),
                            _ap(
                                xt_base,
                                (p0 + dy) * (C * WPAD) + half * WHF,
                                [[C * WPAD, p1 - p0], [WPAD, C], [1, WHF + 4]],
                            ),
                        )
                        if p0 > 0:  # top halo rows <- zeros
                            nc.sync.dma_start(
                                _ap(xtd, 0, [[XF, p0], [1, XF]]), zer.ap()[0:p0, 0:XF]
                            )
                        if p1 < 128:  # bottom halo rows <- zeros
                            nc.sync.dma_start(
                                _ap(xtd, p1 * XF, [[XF, 128 - p1], [1, XF]]),
                                zer.ap()[0 : 128 - p1, 0:XF],
                            )
                        for dx in range(-2, 3):
                            k = (dy + 2) * 5 + (dx + 2)
                            sk = s * K2 + k
                            in0 = _ap(
                                xtd, 2 + dx, [[XF, 128], [WHF + 4, C], [1, WHF]]
                            )
                            in1 = _ap(
                                fr,
                                sk * W + half * WHF,
                                [[SK * W, 128], [0, C], [1, WHF]],
                            )
                            dst3 = _ap(acc, 0, [[C * WHF, 128], [WHF, C], [1, WHF]])
                            if k == 0:
                                nc.vector.tensor_mul(dst3, in0, in1)
                            else:
                                tmp = tmpp.tile([128, C * WHF], F32, tag="tmp")
                                t3 = _ap(tmp, 0, [[C * WHF, 128], [WHF, C], [1, WHF]])
                                nc.vector.tensor_mul(t3, in0, in1)
                                nc.vector.tensor_add(acc[:], acc[:], tmp[:])
                    # reshuffle (c, w) -> (c4, c2, w, c1) and DMA out
                    acc2 = acc2p.tile([128, C * WHF], F32, tag="acc2")
                    nc.scalar.copy(
                        acc2[:].rearrange(
                            "p (a b w d) -> p a b w d", a=16, b=2, w=WHF
                        ),
                        _ap(
                            acc,
                            0,
                            [
                                [C * WHF, 128],
                                [4 * WHF, 16],
                                [2 * WHF, 2],
                                [1, WHF],
                                [WHF, 2],
                            ],
                        ),
                    )
                    # out[s*16+c4, 2h+c2, 2*(half*64+w)+c1]; split per c2
                    for c2 in range(2):
                        dst = bass.AP(
                            tensor=out,
                            offset=(s * 16) * (4 * HW) + c2 * (2 * W) + half * W,
                            ap=[
                                [2 * (2 * W), 128],  # h -> row 2h
                                [4 * HW, 16],  # c4
                                [1, 2 * WHF],  # (w, c1) contiguous
                            ],
                        )
                        src = _ap(
                            acc2,
                            c2 * (2 * WHF),
                            [[C * WHF, 128], [4 * WHF, 16], [1, 2 * WHF]],
                        )
                        nc.sync.dma_start(dst, src)
            acc2p.close()
            tmpp.close()
            accp.close()
            xtp.close()
            pfr.close()
        pp.close()
        cp.close()
    nc.compile()
    return nc, dbg


def host_inputs(x_img, w_compress, b_compress, w_encoder, b_encoder):
    """Per-core input map for one image [C, H, W]."""
    w1t = np.concatenate(
        [w_compress[:, :, 0, 0].T, b_compress[None, :]], axis=0
    ).astype(np.float32)
    wet = np.zeros((M + 1, 9, SK), np.float32)
    for ty in range(3):
        for tx in range(3):
            wet[:M, ty * 3 + tx, :] = w_encoder[:, :, ty, tx].T
    wet[M, 4, :] = b_encoder
    son = np.zeros((SK, S2), np.float32)
    for s in range(S2):
        son[s * K2 : (s + 1) * K2, s] = 1.0
    return {
        "xin": np.ascontiguousarray(x_img.reshape(C, HW)).astype(np.float32),
        "w1t": w1t,
        "wet": wet.reshape(M + 1, 9 * SK),
        "sones": son,
        "onesr": np.ones((1, 130 * 130), np.float32),
        "zer": np.zeros((2, C * WPAD), np.float32),
    }


_CACHE = {}


def kernel(x, w_compress, b_compress, w_encoder, b_encoder):
    x = np.asarray(x, np.float32)
    if "nc" not in _CACHE:
        _CACHE["nc"], _ = build_program(debug=False)
    nc = _CACHE["nc"]
    in_maps = [
        host_inputs(
            x[i],
            np.asarray(w_compress, np.float32),
            np.asarray(b_compress, np.float32),
            np.asarray(w_encoder, np.float32),
            np.asarray(b_encoder, np.float32),
        )
        for i in range(N_CORES)
    ]
    from concourse.bass_utils import run_bass_kernel_spmd

    res = run_bass_kernel_spmd(nc, in_maps, core_ids=list(range(N_CORES)))
    return np.stack([res.results[i]["out"] for i in range(N_CORES)], axis=0)



# revision 9
# speedup vs baseline: 3.3432x; 3.3432x over previous
"""CARAFE kernel for Trainium2 (8 NeuronCores, batch-parallel), v2.

Reference computation per image:
  R = relu(conv1x1(x, w_compress, b_compress))          [48, 128, 128]
  E = conv3x3(R, w_encoder, b_encoder, pad=1)           [100, 128, 128]
  Y = softmax over k of E.reshape(4, 25, H, W)          (s, k, h, w)
  out[s,c,h,w] = sum_k Y[s,k,h,w] * xpad[c, h+dy, w+dx] (k=(dy,dx), 5x5, pad 2)
  pixel-shuffle: out_ref[s*16 + c//4, 2h + (c//2)%2, 2w + c%2] = out[s,c,h,w]

v2 design (vs v1):
  - bf16 everywhere: PE matmuls at 1 cyc/row (vs 4 for fp32), DVE
    tensor_tensor at 2x, all DMA bytes halved.  PSUM accumulation stays
    fp32, exp runs on fp32 PSUM logits.
  - The five dy-shifted pixel-major copies of x (needed because compute
    engines cannot shift partitions) are built on the HOST as a pure
    layout transform and shipped as one [128, 5*C*(W+4)] bf16 input --
    one contiguous DMA instead of 40 SBUF->SBUF copies with 272B
    descriptors (which dominated v1: 328K DMA packets).
  - Softmax normalization folded in per conv block: Z = sones @ F on PE,
    1/Z on DVE, broadcast back over the 25 taps via a second tiny matmul,
    one in-place multiply.  No F DRAM round-trip, no per-(s,w) epilogue.
  - Output leaves the device as raw [s, h, (c,w)] bf16 (4 contiguous 2MB
    DMAs); pixel-shuffle + fp32 cast happen on the host.
"""

import sys

import numpy as np

sys.path.insert(0, "/opt/trn_rl_repo")

import ml_dtypes

import concourse.bass as bass
import concourse.mybir as mybir
import concourse.tile as tile
from concourse import bacc

F32 = mybir.dt.float32
# fp16 (not bf16): the 25-term sequential accumulation in the patch sum
# needs the 10-bit mantissa — bf16's 7 bits put rel-err right at the 2e-2
# gate; fp16 lands at ~2e-3.  Values are O(1-5), far from fp16 range limits.
BF16 = mybir.dt.float16
BF_NP = np.float16

H = 128
W = 128
C = 64
M = 48  # compressed channels
S2 = 4  # scale_factor**2
K2 = 25  # k_up**2
SK = 100
HW = H * W
WPAD = W + 4  # w-padded pixel-major buffers
CW = C * WPAD  # 8448, free elems per (dy) plane
N_CORES = 8
NBLK = HW // 512  # 32 conv blocks of 512 pixels


def _ap(t, extra_off, dims):
    """Raw AP on a tile handle `t` with free-offset `extra_off` (elements)
    and explicit [step, count] dims (dims[0] is the partition dim)."""
    base = t[:]
    return bass.AP(tensor=base.tensor, offset=base.offset + extra_off, ap=dims)


class _Pool:
    """Manually scoped tile pool."""

    def __init__(self, tc, **kw):
        self._cm = tc.tile_pool(**kw)
        self.pool = self._cm.__enter__()
        self._n = 0

    def tile(self, *a, tag=None, **kw):
        self._n += 1
        t = tag or f"t{self._n}"
        return self.pool.tile(*a, tag=t, name=t, **kw)

    def close(self):
        self._cm.__exit__(None, None, None)


def build_program():
    nc = bacc.Bacc("TRN2", target_bir_lowering=False, debug=False)

    xc = nc.dram_tensor("xc", [C + 1, HW], BF16, kind="ExternalInput")
    xt5d = nc.dram_tensor("xt5", [128, 5 * CW], BF16, kind="ExternalInput")
    w1t = nc.dram_tensor("w1t", [C + 1, M], BF16, kind="ExternalInput")
    wet = nc.dram_tensor("wet", [M + 1, 9 * SK], BF16, kind="ExternalInput")
    sones = nc.dram_tensor("sones", [SK, S2], BF16, kind="ExternalInput")
    sonesT = nc.dram_tensor("sonesT", [S2, SK], BF16, kind="ExternalInput")
    identd = nc.dram_tensor("ident", [SK, SK], BF16, kind="ExternalInput")
    onesr = nc.dram_tensor("onesr", [1, 130 * 130], BF16, kind="ExternalInput")
    out = nc.dram_tensor("out", [S2, H, C * W], BF16, kind="ExternalOutput")

    with tile.TileContext(nc) as tc:
        cp = _Pool(tc, name="consts", bufs=1)
        w1t_sb = cp.tile([C + 1, M], BF16)
        nc.sync.dma_start(w1t_sb[:], w1t.ap())
        wet_sb = cp.tile([M + 1, 9 * SK], BF16)
        nc.sync.dma_start(wet_sb[:], wet.ap())
        sones_sb = cp.tile([SK, S2], BF16)
        nc.sync.dma_start(sones_sb[:], sones.ap())
        sonesT_sb = cp.tile([S2, SK], BF16)
        nc.sync.dma_start(sonesT_sb[:], sonesT.ap())
        ident_sb = cp.tile([SK, SK], BF16)
        nc.sync.dma_start(ident_sb[:], identd.ap())

        pp = _Pool(tc, name="persist", bufs=1)
        xt5 = pp.tile([128, 5 * CW], BF16)
        nc.sync.dma_start(xt5[:], xt5d.ap())
        fr = pp.tile([128, SK * W], BF16)

        p2 = _Pool(tc, name="fnorm", bufs=1)
        f_norm = p2.tile([SK, HW], BF16)

        # ---- conv1x1 -> relu -> R_pad (full image, 1-halo borders) ----
        p3 = _Pool(tc, name="rpad", bufs=1)
        r_pad = p3.tile([M + 1, 130 * 130], BF16)
        nc.vector.memset(r_pad[:], 0.0)
        nc.sync.dma_start(
            _ap(r_pad, M * 130 * 130, [[130 * 130, 1], [1, 130 * 130]]), onesr.ap()
        )

        p4 = _Pool(tc, name="xb", bufs=2)
        psA = _Pool(tc, name="psA", bufs=2, space="PSUM")
        CHUNK = 8  # conv1x1 blocks per x chunk-load
        for jc in range(NBLK // CHUNK):
            xb = p4.tile([C + 1, 512 * CHUNK], BF16, tag="xb")
            nc.sync.dma_start(
                xb[:], xc.ap()[:, jc * 512 * CHUNK : (jc + 1) * 512 * CHUNK]
            )
            for ji in range(CHUNK):
                j = jc * CHUNK + ji
                ps1 = psA.tile([M, 512], F32, tag="ps1")
                nc.tensor.matmul(
                    ps1[:],
                    w1t_sb[:],
                    xb[:, ji * 512 : (ji + 1) * 512],
                    start=True,
                    stop=True,
                )
                nc.scalar.activation(
                    _ap(
                        r_pad,
                        (1 + 4 * j) * 130 + 1,
                        [[130 * 130, M], [130, 4], [1, W]],
                    ),
                    ps1[:],
                    mybir.ActivationFunctionType.Relu,
                )
        psA.close()
        p4.close()

        # ---- conv3x3 -> exp -> normalized F (channel-major, bf16) ----
        psB = _Pool(tc, name="psB", bufs=4, space="PSUM")
        psC = _Pool(tc, name="psC", bufs=2, space="PSUM")
        psD = _Pool(tc, name="psD", bufs=2, space="PSUM")
        rzp = _Pool(tc, name="rz", bufs=2)
        for j in range(NBLK):
            ps2 = psB.tile([SK, 512], F32, tag="ps2")
            for t in range(9):
                ty, tx = divmod(t, 3)
                nc.tensor.matmul(
                    ps2[:],
                    wet_sb[:, t * SK : (t + 1) * SK],
                    _ap(r_pad, (4 * j + ty) * 130 + tx, [[130 * 130, M + 1], [130, 4], [1, W]]),
                    start=(t == 0),
                    stop=(t == 8),
                )
            fblk = f_norm[:, j * 512 : (j + 1) * 512]
            nc.scalar.activation(fblk, ps2[:], mybir.ActivationFunctionType.Exp)
            psz = psC.tile([S2, 512], F32, tag="psz")
            nc.tensor.matmul(psz[:], sones_sb[:], fblk, start=True, stop=True)
            rz = rzp.tile([S2, 512], BF16, tag="rz")
            with nc.allow_low_precision(reason="softmax weights tolerate bf16"):
                nc.vector.reciprocal(rz[:], psz[:])
            zb = psD.tile([SK, 512], F32, tag="zb")
            nc.tensor.matmul(zb[:], sonesT_sb[:], rz[:], start=True, stop=True)
            nc.vector.tensor_mul(fblk, fblk, zb[:])
        rzp.close()
        psD.close()
        psC.close()
        psB.close()
        p3.close()

        # ---- F^T transposes -> FR [128(h), (sk, w)] ----
        psF = _Pool(tc, name="psF", bufs=4, space="PSUM")
        for w in range(W):
            pst = psF.tile([128, SK], BF16, tag="pst")
            nc.tensor.transpose(
                pst[:], _ap(f_norm, w, [[HW, SK], [W, H]]), ident_sb[:]
            )
            nc.scalar.copy(_ap(fr, w, [[SK * W, 128], [W, SK]]), pst[:])
        psF.close()
        p2.close()

        # ---- per-pixel patch sum on VectorE; out[s] = [128(h), (c, w)] ----
        pacc = _Pool(tc, name="acc", bufs=2)
        ptmp = _Pool(tc, name="tmp", bufs=2)
        for s in range(S2):
            acc = pacc.tile([128, C * W], BF16, tag="acc")
            for dy in range(-2, 3):
                for dx in range(-2, 3):
                    k = (dy + 2) * 5 + (dx + 2)
                    sk = s * K2 + k
                    in0 = _ap(
                        xt5,
                        (dy + 2) * CW + 2 + dx,
                        [[5 * CW, 128], [WPAD, C], [1, W]],
                    )
                    in1 = _ap(fr, sk * W, [[SK * W, 128], [0, C], [1, W]])
                    dst3 = _ap(acc, 0, [[C * W, 128], [W, C], [1, W]])
                    if k == 0:
                        nc.vector.tensor_mul(dst3, in0, in1)
                    else:
                        tmp = ptmp.tile([128, C * W], BF16, tag="tmp")
                        t3 = _ap(tmp, 0, [[C * W, 128], [W, C], [1, W]])
                        nc.vector.tensor_mul(t3, in0, in1)
                        nc.vector.tensor_add(acc[:], acc[:], tmp[:])
            nc.sync.dma_start(
                bass.AP(tensor=out, offset=s * H * C * W, ap=[[C * W, 128], [1, C * W]]),
                acc[:],
            )
        ptmp.close()
        pacc.close()
        pp.close()
        cp.close()
    nc.compile()
    return nc


def host_inputs(x_img, w_compress, b_compress, w_encoder, b_encoder):
    """Per-core input map for one image [C, H, W] (all bf16)."""
    x_img = np.asarray(x_img, np.float32)
    xc = np.concatenate(
        [x_img.reshape(C, HW), np.ones((1, HW), np.float32)], axis=0
    ).astype(BF_NP)
    # pixel-major, w-padded, 5 dy-shifted planes: xt5[h, dy, c, wp]
    #   = xpad[c, h + dy, wp]  (xpad has pad 2 on h and w)
    xpad = np.pad(x_img, ((0, 0), (2, 2), (2, 2))).astype(BF_NP)
    xt5 = np.stack([xpad[:, dy : dy + H, :] for dy in range(5)], axis=0)
    xt5 = np.ascontiguousarray(xt5.transpose(2, 0, 1, 3)).reshape(128, 5 * CW)
    w1t = np.concatenate(
        [w_compress[:, :, 0, 0].T, b_compress[None, :]], axis=0
    ).astype(BF_NP)
    wetm = np.zeros((M + 1, 9, SK), np.float32)
    for ty in range(3):
        for tx in range(3):
            wetm[:M, ty * 3 + tx, :] = w_encoder[:, :, ty, tx].T
    wetm[M, 4, :] = b_encoder
    son = np.zeros((SK, S2), np.float32)
    for s in range(S2):
        son[s * K2 : (s + 1) * K2, s] = 1.0
    return {
        "xc": xc,
        "xt5": xt5,
        "w1t": w1t,
        "wet": wetm.reshape(M + 1, 9 * SK).astype(BF_NP),
        "sones": son.astype(BF_NP),
        "sonesT": np.ascontiguousarray(son.T).astype(BF_NP),
        "ident": np.eye(SK, dtype=BF_NP),
        "onesr": np.ones((1, 130 * 130), BF_NP),
    }


def _unshuffle(dev_out):
    """[S2, H, C*W] bf16 -> [64, 256, 256] fp32 pixel-shuffled output."""
    a = np.asarray(dev_out).reshape(S2, H, 16, 2, 2, W)  # s, h, c4, c2, c1, w
    a = a.transpose(0, 2, 1, 3, 5, 4)  # s, c4, h, c2, w, c1
    return np.ascontiguousarray(a).reshape(C, 2 * H, 2 * W).astype(np.float32)


_CACHE = {}


def kernel(x, w_compress, b_compress, w_encoder, b_encoder):
    x = np.asarray(x, np.float32)
    if "nc" not in _CACHE:
        _CACHE["nc"] = build_program()
    nc = _CACHE["nc"]
    in_maps = [
        host_inputs(
            x[i],
            np.asarray(w_compress, np.float32),
            np.asarray(b_compress, np.float32),
            np.asarray(w_encoder, np.float32),
            np.asarray(b_encoder, np.float32),
        )
        for i in range(N_CORES)
    ]
    from concourse.bass_utils import run_bass_kernel_spmd

    res = run_bass_kernel_spmd(nc, in_maps, core_ids=list(range(N_CORES)))
    return np.stack(
        [_unshuffle(res.results[i]["out"]) for i in range(N_CORES)], axis=0
    )
